# revision 7
# baseline (speedup 1.0000x reference)
"""Dense2DSpatialTransformer (bilinear warp with N(0,1) flow) on 8 TRN2 cores.

Strategy (per spec sharding hint): embarrassingly data-parallel over batch —
each of the 8 cores processes 2 of the 16 images independently.

Per-core dense algorithm ("tent weight" formulation):
  bilinear sampling at (h + dH, w + dW) is rewritten exactly as
      out[h,w] = sum_j tent(R_H - j) * sum_k tent(R_W - k) * img[h+j+1, w+k+1]
  with tent(x) = relu(1 - |x|), R_H = dH, R_W = dW, and j,k ranging over
  the integer window [-WIN, WIN].  The tent weights are continuous in R,
  so no floor/mask consistency with the host is needed and all value math
  can run in fp16 (error is O(ulp * local image gradient), far below the
  2e-2 gate).  Per 128x1024 tile:
    * Activation engine builds the 2*(2*WIN+1) tent-weight planes
      (Abs + Relu activation pairs) from the fp16 flow.
    * DVE + Pool (gpsimd) engines compute the 2*WIN+1 horizontally-lerped
      candidate rows (9 mult + 8 add each, fp16 tensor_tensor at 2x rate),
      then DVE combines them with the vertical tent weights.
  Edge clipping is realized by an 8-wide replicate-padded fp16 copy of the
  image built in DRAM (replicate-pad == the reference's index clipping).

Sparse fixup: pixels with |R| >= WIN - margin (~1 in 8000 for WIN=4) are
computed exactly on device via per-element indirect-DMA gathers from the
padded image and scattered into the output.  Host only supplies their
positions (index metadata derived from the inputs at call time).
"""
import sys

for _p in ("/opt/trn_rl_repo", "/opt/trn_rl_repo/concourse",
           "/root/.axon_site/_ro/trn_rl_repo"):
    if _p not in sys.path:
        sys.path.insert(0, _p)

import numpy as np

import concourse.bass as bass
import concourse.bacc as bacc
import concourse.mybir as mybir
import concourse.tile as tile
from concourse.bass import IndirectOffsetOnAxis
from concourse.bass_utils import run_bass_kernel_spmd

f32 = np.float32
FP = mybir.dt.float32
F16 = mybir.dt.float16
I32 = mybir.dt.int32

B, H, W = 16, 1024, 1024
NCORES = 8
BPC = B // NCORES           # images per core
PAD = 8
PP = H + 2 * PAD            # padded image side (1040)
WIN = 4                     # tent half-window; dense covers |R| <= WIN
NTAP = 2 * WIN + 1
F = 1024                    # free-dim tile width (full row)
NROW = H // 128             # row blocks per image
HW = H * W
OUT_TAIL = 128              # scratch tail for fixup padding writes
MARGIN = f32(0.01)          # host outlier classification guard band
POOL_ROWS = (-WIN, WIN)     # candidate rows computed on the gpsimd engine
FLOOR_LO, FLOOR_HI = -7, 6  # fixup floor cascade range

AL = mybir.AluOpType
AF = mybir.ActivationFunctionType


def _build_program(nout):
    nc = bacc.Bacc("TRN2", target_bir_lowering=False, debug=False,
                   enable_asserts=False, num_devices=NCORES)

    img_d = nc.dram_tensor("img", [BPC, H, W], FP, kind="ExternalInput")
    flow_d = nc.dram_tensor("flow", [BPC * 2 * HW], FP, kind="ExternalInput")
    opos_d = nc.dram_tensor("opos", [nout], I32, kind="ExternalInput")
    odh_d = nc.dram_tensor("odh", [nout], I32, kind="ExternalInput")
    odw_d = nc.dram_tensor("odw", [nout], I32, kind="ExternalInput")
    oh_d = nc.dram_tensor("oh", [nout], FP, kind="ExternalInput")
    ow_d = nc.dram_tensor("ow", [nout], FP, kind="ExternalInput")
    obase_d = nc.dram_tensor("obase", [nout], FP, kind="ExternalInput")
    ppad_d = nc.dram_tensor("ppad", [BPC * PP * PP], F16, kind="Internal")
    out_d = nc.dram_tensor("out", [BPC * HW + OUT_TAIL], FP,
                           kind="ExternalOutput")

    img = img_d.ap()
    flowf = flow_d.ap()
    flow4 = flowf.rearrange("(b c h w) -> b c h w", b=BPC, c=2, h=H, w=W)
    ppf = ppad_d.ap()
    pp3 = ppf.rearrange("(b h w) -> b h w", b=BPC, h=PP, w=PP)
    outf = out_d.ap()
    out3 = outf[0:BPC * HW].rearrange("(b h w) -> b h w", b=BPC, h=H, w=W)

    v = nc.vector
    g = nc.gpsimd
    a = nc.scalar

    with tile.TileContext(nc) as tc:
        # ---- phase 0: build fp16 replicate-padded images in DRAM ----
        with tc.tile_pool(name="pad", bufs=2) as pd:
            for b in range(BPC):
                for i in range(NROW):
                    r0 = 128 * i
                    T = pd.tile([128, W], FP, tag="padT")
                    nc.sync.dma_start(out=T[:], in_=img[b, r0:r0 + 128, :])
                    P = pd.tile([128, PP], F16, tag="padP")
                    v.tensor_copy(out=P[:, PAD:PAD + W], in_=T[:])
                    v.tensor_copy(out=P[:, 0:PAD],
                                  in_=T[:, 0:1].broadcast_to([128, PAD]))
                    v.tensor_copy(out=P[:, PAD + W:PP],
                                  in_=T[:, W - 1:W].broadcast_to([128, PAD]))
                    nc.sync.dma_start(
                        out=pp3[b, PAD + r0:PAD + r0 + 128, :], in_=P[:])
            for b in range(BPC):
                for k in range(PAD):
                    nc.sync.dma_start(out=pp3[b, k:k + 1, :],
                                      in_=pp3[b, PAD:PAD + 1, :])
                    nc.sync.dma_start(out=pp3[b, PAD + H + k:PAD + H + k + 1, :],
                                      in_=pp3[b, PAD + H - 1:PAD + H, :])

        # ---- dense tiles ----
        with tc.tile_pool(name="pers", bufs=1) as pers, \
             tc.tile_pool(name="io", bufs=2) as io, \
             tc.tile_pool(name="wq", bufs=2) as wq, \
             tc.tile_pool(name="wk", bufs=2) as wk:
            bias_ap = {}
            for k in range(-WIN, WIN + 1):
                bt = pers.tile([128, 1], FP, tag=f"bias{k}")
                g.memset(bt[:], float(-k))
                bias_ap[k] = bt
            for b in range(BPC):
                for i in range(NROW):
                    r0 = 128 * i
                    imgS = {}
                    for j in range(-WIN, WIN + 1):
                        t_img = io.tile([128, W + 2 * WIN], F16, tag=f"imgS{j}")
                        nc.sync.dma_start(
                            out=t_img[:],
                            in_=pp3[b, r0 + PAD + j:r0 + PAD + j + 128,
                                    PAD - WIN:PAD + WIN + W])
                        imgS[j] = t_img
                    dH = io.tile([128, F], FP, tag="dH")
                    nc.sync.dma_start(out=dH[:], in_=flow4[b, 0, r0:r0 + 128, :])
                    dW = io.tile([128, F], FP, tag="dW")
                    nc.sync.dma_start(out=dW[:], in_=flow4[b, 1, r0:r0 + 128, :])

                    # fp16 flow (Act engine)
                    Rh = wq.tile([128, F], F16, tag="Rh")
                    a.activation(out=Rh[:], in_=dH[:], func=AF.Copy)
                    Rw = wq.tile([128, F], F16, tag="Rw")
                    a.activation(out=Rw[:], in_=dW[:], func=AF.Copy)

                    # tent weights (Act engine): q = relu(1 - |R - k|)
                    def tents(R_t, pfx):
                        q = {}
                        for k in range(-WIN, WIN + 1):
                            u = wq.tile([128, F], F16, tag=f"{pfx}u")
                            a.activation(out=u[:], in_=R_t[:], func=AF.Abs,
                                         bias=bias_ap[k][:, 0:1], scale=1.0)
                            qk = wq.tile([128, F], F16, tag=f"{pfx}q{k}")
                            a.activation(out=qk[:], in_=u[:], func=AF.Relu,
                                         bias=1.0, scale=-1.0)
                            q[k] = qk
                        return q

                    qW = tents(Rw, "w")
                    qH = tents(Rh, "h")

                    # horizontally-lerped candidate rows
                    HL = {}
                    for j in range(-WIN, WIN + 1):
                        eng = g if j in POOL_ROWS else v
                        hl = wk.tile([128, F], F16, tag=f"HL{j}")
                        eng.tensor_tensor(out=hl[:], in0=qW[-WIN][:],
                                          in1=imgS[j][:, 0:F], op=AL.mult)
                        for k in range(-WIN + 1, WIN + 1):
                            tk = wk.tile([128, F], F16,
                                         tag="tkg" if eng is g else "tkv")
                            eng.tensor_tensor(
                                out=tk[:], in0=qW[k][:],
                                in1=imgS[j][:, k + WIN:k + WIN + F], op=AL.mult)
                            eng.tensor_tensor(out=hl[:], in0=hl[:], in1=tk[:],
                                              op=AL.add)
                        HL[j] = hl

                    # vertical tent combine (DVE)
                    vacc = wk.tile([128, F], F16, tag="vacc")
                    v.tensor_tensor(out=vacc[:], in0=qH[-WIN][:],
                                    in1=HL[-WIN][:], op=AL.mult)
                    outT = wk.tile([128, F], FP, tag="outT")
                    for j in range(-WIN + 1, WIN + 1):
                        tv = wk.tile([128, F], F16, tag="tkv")
                        v.tensor_tensor(out=tv[:], in0=qH[j][:], in1=HL[j][:],
                                        op=AL.mult)
                        if j < WIN:
                            v.tensor_tensor(out=vacc[:], in0=vacc[:],
                                            in1=tv[:], op=AL.add)
                        else:
                            v.tensor_tensor(out=outT[:], in0=vacc[:],
                                            in1=tv[:], op=AL.add)
                    nc.sync.dma_start(out=out3[b, r0:r0 + 128, :], in_=outT[:])

        # ---- sparse fixup ----
        # offsets [128, 1] (one per partition), one descriptor per partition;
        # outliers processed in chunks of 128 with single-element rows; field
        # math vectorized across chunks.
        NCH = nout // 128
        with tc.tile_pool(name="fix", bufs=1) as fx:
            def load_aux(d, dt, name):
                t = fx.tile([128, NCH], dt, tag=name)
                nc.sync.dma_start(
                    out=t[:], in_=d.ap().rearrange("(p f) -> p f", p=128))
                return t

            opos_s = load_aux(opos_d, I32, "opos")
            odh_s = load_aux(odh_d, I32, "odh")
            odw_s = load_aux(odw_d, I32, "odw")
            oh_s = load_aux(oh_d, FP, "oh")
            ow_s = load_aux(ow_d, FP, "ow")
            obase_s = load_aux(obase_d, FP, "obase")

            dhv = fx.tile([128, NCH], FP, tag="dhv")
            dwv = fx.tile([128, NCH], FP, tag="dwv")
            for c in range(NCH):
                g.indirect_dma_start(
                    out=dhv[:, c:c + 1], out_offset=None,
                    in_=flowf[:, None],
                    in_offset=IndirectOffsetOnAxis(ap=odh_s[:, c:c + 1], axis=0))
                g.indirect_dma_start(
                    out=dwv[:, c:c + 1], out_offset=None,
                    in_=flowf[:, None],
                    in_offset=IndirectOffsetOnAxis(ap=odw_s[:, c:c + 1], axis=0))

            def floor_frac(dv, pfx):
                """floor(R) and (floor(R)+1) - R over R in [FLOOR_LO, FLOOR_HI+1)."""
                St = fx.tile([128, NCH], FP, tag=f"{pfx}S")
                gt = fx.tile([128, NCH], FP, tag=f"{pfx}g")
                v.tensor_scalar(out=St[:], in0=dv[:],
                                scalar1=float(FLOOR_LO + 1), scalar2=None,
                                op0=AL.is_ge)
                for s in range(FLOOR_LO + 2, FLOOR_HI + 1):
                    v.tensor_scalar(out=gt[:], in0=dv[:], scalar1=float(s),
                                    scalar2=None, op0=AL.is_ge)
                    v.tensor_tensor(out=St[:], in0=St[:], in1=gt[:], op=AL.add)
                # St = sum of indicators -> floor = St + FLOOR_LO
                fl = fx.tile([128, NCH], FP, tag=f"{pfx}fl")
                v.tensor_scalar(out=fl[:], in0=St[:], scalar1=float(FLOOR_LO),
                                scalar2=None, op0=AL.add)
                dd = fx.tile([128, NCH], FP, tag=f"{pfx}dd")
                # dd = (floor + 1) - R
                v.tensor_scalar(out=dd[:], in0=fl[:], scalar1=1.0,
                                scalar2=None, op0=AL.add)
                v.tensor_tensor(out=dd[:], in0=dd[:], in1=dv[:], op=AL.subtract)
                return fl, dd

            flh, ddh = floor_frac(dhv, "fh")
            flw, ddw = floor_frac(dwv, "fw")

            # addr = obase + (oh + floor_h + PAD)*PP + (ow + floor_w + PAD)
            rowp = fx.tile([128, NCH], FP, tag="rowp")
            v.tensor_tensor(out=rowp[:], in0=oh_s[:], in1=flh[:], op=AL.add)
            v.tensor_scalar(out=rowp[:], in0=rowp[:], scalar1=float(PAD),
                            scalar2=float(PP), op0=AL.add, op1=AL.mult)
            colp = fx.tile([128, NCH], FP, tag="colp")
            v.tensor_tensor(out=colp[:], in0=ow_s[:], in1=flw[:], op=AL.add)
            v.tensor_scalar(out=colp[:], in0=colp[:], scalar1=float(PAD),
                            scalar2=None, op0=AL.add)
            af = fx.tile([128, NCH], FP, tag="af")
            v.tensor_tensor(out=af[:], in0=rowp[:], in1=colp[:], op=AL.add)
            v.tensor_tensor(out=af[:], in0=af[:], in1=obase_s[:], op=AL.add)

            vals = {}
            afo = fx.tile([128, NCH], FP, tag="afo")
            for (cn, doff) in (("v00", 0.0), ("v10", 1.0),
                               ("v01", float(PP)), ("v11", float(PP + 1))):
                ai = fx.tile([128, NCH], I32, tag=f"ai{cn}")
                if doff == 0.0:
                    v.tensor_copy(out=ai[:], in_=af[:])
                else:
                    v.tensor_scalar(out=afo[:], in0=af[:], scalar1=doff,
                                    scalar2=None, op0=AL.add)
                    v.tensor_copy(out=ai[:], in_=afo[:])
                vt16 = fx.tile([128, NCH], F16, tag=f"{cn}h")
                for c in range(NCH):
                    g.indirect_dma_start(
                        out=vt16[:, c:c + 1], out_offset=None,
                        in_=ppf[:, None],
                        in_offset=IndirectOffsetOnAxis(ap=ai[:, c:c + 1],
                                                       axis=0))
                vt = fx.tile([128, NCH], FP, tag=cn)
                v.tensor_copy(out=vt[:], in_=vt16[:])
                vals[cn] = vt

            # blend: out = v00*dh*dw + v10*dh*(1-dw) + v01*(1-dh)*dw
            #            + v11*(1-dw)*(1-dh)   with dh=ddh, dw=ddw
            omw = fx.tile([128, NCH], FP, tag="omw")
            v.tensor_scalar(out=omw[:], in0=ddw[:], scalar1=-1.0, scalar2=1.0,
                            op0=AL.mult, op1=AL.add)
            omh = fx.tile([128, NCH], FP, tag="omh")
            v.tensor_scalar(out=omh[:], in0=ddh[:], scalar1=-1.0, scalar2=1.0,
                            op0=AL.mult, op1=AL.add)
            wt = fx.tile([128, NCH], FP, tag="wtf")
            accf = fx.tile([128, NCH], FP, tag="accf")
            t3 = fx.tile([128, NCH], FP, tag="t3")
            v.tensor_tensor(out=wt[:], in0=ddh[:], in1=ddw[:], op=AL.mult)
            v.tensor_tensor(out=accf[:], in0=vals["v00"][:], in1=wt[:],
                            op=AL.mult)
            v.tensor_tensor(out=wt[:], in0=ddh[:], in1=omw[:], op=AL.mult)
            v.tensor_tensor(out=t3[:], in0=vals["v10"][:], in1=wt[:], op=AL.mult)
            v.tensor_tensor(out=accf[:], in0=accf[:], in1=t3[:], op=AL.add)
            v.tensor_tensor(out=wt[:], in0=omh[:], in1=ddw[:], op=AL.mult)
            v.tensor_tensor(out=t3[:], in0=vals["v01"][:], in1=wt[:], op=AL.mult)
            v.tensor_tensor(out=accf[:], in0=accf[:], in1=t3[:], op=AL.add)
            v.tensor_tensor(out=wt[:], in0=omw[:], in1=omh[:], op=AL.mult)
            v.tensor_tensor(out=t3[:], in0=vals["v11"][:], in1=wt[:], op=AL.mult)
            v.tensor_tensor(out=accf[:], in0=accf[:], in1=t3[:], op=AL.add)

            for c in range(NCH):
                g.indirect_dma_start(
                    out=outf[:, None],
                    out_offset=IndirectOffsetOnAxis(ap=opos_s[:, c:c + 1],
                                                    axis=0),
                    in_=accf[:, c:c + 1], in_offset=None)

    nc.compile()
    return nc


_PROGRAM_CACHE = {}


def _get_program(nout):
    if nout not in _PROGRAM_CACHE:
        _PROGRAM_CACHE[nout] = _build_program(nout)
    return _PROGRAM_CACHE[nout]


def _host_metadata(dH, dW):
    """Outlier positions for one image, mirroring the reference fp32 math."""
    h = (np.arange(H, dtype=f32)[:, None] * np.ones((1, W), f32))
    w = (np.ones((H, 1), f32) * np.arange(W, dtype=f32)[None, :])
    Rh = (((dH + h).astype(f32) + f32(1.0)).astype(f32)
          - (h + f32(1.0)).astype(f32)).astype(f32)
    Rw = (((dW + w).astype(f32) + f32(1.0)).astype(f32)
          - (w + f32(1.0)).astype(f32)).astype(f32)
    lim = f32(WIN) - MARGIN
    outl = (np.abs(Rh) >= lim) | (np.abs(Rw) >= lim)
    oy, ox = np.where(outl)
    return oy.astype(np.int64), ox.astype(np.int64)


def _prepare(input1, input2):
    """Build (or fetch) the program and the per-core input maps."""
    input1 = np.asarray(input1)
    input2 = np.asarray(input2)
    assert input1.shape == (B, 1, H, W) and input2.shape == (B, 2, H, W)

    metas = []
    max_n = 1
    for c in range(NCORES):
        rows = []
        for bl in range(BPC):
            bglob = c * BPC + bl
            oy, ox = _host_metadata(input2[bglob, 0], input2[bglob, 1])
            rows.append((bl, oy, ox))
        n = sum(len(oy) for _, oy, _ in rows)
        max_n = max(max_n, n)
        metas.append(rows)
    nout = max(128, ((max_n + 127) // 128) * 128)

    nc = _get_program(nout)

    in_maps = []
    for c in range(NCORES):
        imgs = input1[c * BPC:(c + 1) * BPC, 0]
        flow = input2[c * BPC:(c + 1) * BPC]
        opos = np.full(nout, BPC * HW, np.int32)
        odh = np.zeros(nout, np.int32)
        odw = np.full(nout, HW, np.int32)
        oh = np.zeros(nout, f32)
        ow = np.zeros(nout, f32)
        obase = np.zeros(nout, f32)
        k = 0
        for bl, oy, ox in metas[c]:
            n = len(oy)
            opos[k:k + n] = (bl * HW + oy * W + ox).astype(np.int32)
            odh[k:k + n] = (bl * 2 * HW + oy * W + ox).astype(np.int32)
            odw[k:k + n] = (bl * 2 * HW + HW + oy * W + ox).astype(np.int32)
            oh[k:k + n] = oy.astype(f32)
            ow[k:k + n] = ox.astype(f32)
            obase[k:k + n] = f32(bl * PP * PP)
            k += n
        in_maps.append({
            "img": np.ascontiguousarray(imgs),
            "flow": np.ascontiguousarray(flow.reshape(-1)),
            "opos": opos, "odh": odh, "odw": odw,
            "oh": oh, "ow": ow, "obase": obase,
        })

    return nc, in_maps


def _assemble(results):
    out = np.empty((B, 1, H, W), f32)
    for c in range(NCORES):
        o = results[c]["out"][:BPC * HW].reshape(BPC, H, W)
        out[c * BPC:(c + 1) * BPC, 0] = o
    return out


def kernel(input1, input2):
    nc, in_maps = _prepare(input1, input2)
    res = run_bass_kernel_spmd(nc, in_maps, core_ids=list(range(NCORES)))
    return _assemble(res.results)


# revision 13
# speedup vs baseline: 1.3141x; 1.3141x over previous
"""Dense2DSpatialTransformer (bilinear warp with N(0,1) flow) on 8 TRN2 cores.

Strategy (per spec sharding hint): embarrassingly data-parallel over batch —
each of the 8 cores processes 2 of the 16 images independently.

Per-core dense algorithm ("tent weight" formulation):
  bilinear sampling at (h + dH, w + dW) is rewritten exactly as
      out[h,w] = sum_j tent(R_H - j) * sum_k tent(R_W - k) * img[h+j+1, w+k+1]
  with tent(x) = relu(1 - |x|), R_H = dH, R_W = dW, and j,k over the integer
  window [-WIN, WIN].  Tent weights are continuous in R, so no floor/mask
  consistency with the host is needed and all value math runs in bf16
  (error is O(ulp * local image gradient), below the 2e-2 gate).
  Per 128x1024 tile:
    * Activation engine builds the 2*(2*WIN+1) tent-weight planes
      (Abs + Relu activation pairs) straight from the fp32 flow.
    * DVE computes 2*WIN horizontally-lerped candidate rows as bf16
      tensor_tensor mult/add chains (2x DVE rate: 16-bit packed + 4B-aligned
      operands — image rows are loaded at both column parities so every tap
      slice starts 4B-aligned); the gpsimd engine takes one row.
    * DVE combines rows with the vertical tent weights; final add in fp32.
  Edge clipping is realized by an 8-wide replicate-padded bf16 copy of the
  image built in DRAM (replicate-pad == the reference's index clipping).

Sparse fixup: pixels with |R| >= WIN - margin (~0.3% for WIN=3) are computed
exactly on device via batched indirect-DMA gathers from the padded image and
scattered into the output.  Host only supplies their positions (index
metadata derived from the inputs at call time).
"""
import sys

for _p in ("/opt/trn_rl_repo", "/opt/trn_rl_repo/concourse",
           "/root/.axon_site/_ro/trn_rl_repo"):
    if _p not in sys.path:
        sys.path.insert(0, _p)

import numpy as np

import concourse.bass as bass
import concourse.bacc as bacc
import concourse.mybir as mybir
import concourse.tile as tile
from concourse.bass import IndirectOffsetOnAxis
from concourse.bass_utils import run_bass_kernel_spmd

f32 = np.float32
FP = mybir.dt.float32
BF = mybir.dt.bfloat16
I32 = mybir.dt.int32

B, H, W = 16, 1024, 1024
NCORES = 8
BPC = B // NCORES           # images per core
PAD = 8
PP = H + 2 * PAD            # padded image side (1040)
WIN = 3                     # tent half-window; dense covers |R| <= WIN
F = 1024                    # free-dim tile width (full row)
NROW = H // 128             # row blocks per image
HW = H * W
OUT_TAIL = 128              # scratch tail for fixup padding writes
MARGIN = f32(0.01)          # host outlier classification guard band
POOL_ROWS = ()              # gpsimd does the fixup instead of dense rows
FLOOR_LO, FLOOR_HI = -7, 6  # fixup floor cascade range

AL = mybir.AluOpType
AF = mybir.ActivationFunctionType


def _build_program(nout):
    nc = bacc.Bacc("TRN2", target_bir_lowering=False, debug=False,
                   enable_asserts=False, num_devices=NCORES)

    img_d = nc.dram_tensor("img", [BPC, H, W], FP, kind="ExternalInput")
    flow_d = nc.dram_tensor("flow", [BPC * 2 * HW], FP, kind="ExternalInput")
    opos_d = nc.dram_tensor("opos", [nout], I32, kind="ExternalInput")
    odh_d = nc.dram_tensor("odh", [nout], I32, kind="ExternalInput")
    odw_d = nc.dram_tensor("odw", [nout], I32, kind="ExternalInput")
    oh_d = nc.dram_tensor("oh", [nout], FP, kind="ExternalInput")
    ow_d = nc.dram_tensor("ow", [nout], FP, kind="ExternalInput")
    obase_d = nc.dram_tensor("obase", [nout], FP, kind="ExternalInput")
    ppad_d = nc.dram_tensor("ppad", [BPC * PP * PP], BF, kind="Internal")
    out_d = nc.dram_tensor("out", [BPC * HW + OUT_TAIL], FP,
                           kind="ExternalOutput")

    img = img_d.ap()
    flowf = flow_d.ap()
    flow4 = flowf.rearrange("(b c h w) -> b c h w", b=BPC, c=2, h=H, w=W)
    ppf = ppad_d.ap()
    pp3 = ppf.rearrange("(b h w) -> b h w", b=BPC, h=PP, w=PP)
    outf = out_d.ap()
    out3 = outf[0:BPC * HW].rearrange("(b h w) -> b h w", b=BPC, h=H, w=W)

    v = nc.vector
    g = nc.gpsimd
    a = nc.scalar

    with tile.TileContext(nc) as tc:
        # ---- phase 0: build bf16 replicate-padded images in DRAM ----
        with tc.tile_pool(name="pad", bufs=2) as pd:
            for b in range(BPC):
                for i in range(NROW):
                    r0 = 128 * i
                    T = pd.tile([128, W], FP, tag="padT")
                    nc.sync.dma_start(out=T[:], in_=img[b, r0:r0 + 128, :])
                    P = pd.tile([128, PP], BF, tag="padP")
                    v.tensor_copy(out=P[:, PAD:PAD + W], in_=T[:])
                    v.tensor_copy(out=P[:, 0:PAD],
                                  in_=T[:, 0:1].broadcast_to([128, PAD]))
                    v.tensor_copy(out=P[:, PAD + W:PP],
                                  in_=T[:, W - 1:W].broadcast_to([128, PAD]))
                    nc.sync.dma_start(
                        out=pp3[b, PAD + r0:PAD + r0 + 128, :], in_=P[:])
            for b in range(BPC):
                for k in range(PAD):
                    nc.sync.dma_start(out=pp3[b, k:k + 1, :],
                                      in_=pp3[b, PAD:PAD + 1, :])
                    nc.sync.dma_start(out=pp3[b, PAD + H + k:PAD + H + k + 1, :],
                                      in_=pp3[b, PAD + H - 1:PAD + H, :])

        # ---- dense tiles ----
        with tc.tile_pool(name="pers", bufs=1) as pers, \
             tc.tile_pool(name="io", bufs=2) as io, \
             tc.tile_pool(name="wq", bufs=2) as wq, \
             tc.tile_pool(name="wk", bufs=2) as wk:
            bias_ap = {}
            for k in range(-WIN, WIN + 1):
                bt = pers.tile([128, 1], FP, tag=f"bias{k}")
                g.memset(bt[:], float(-k))
                bias_ap[k] = bt
            for b in range(BPC):
                for i in range(NROW):
                    r0 = 128 * i
                    # two column parities so every tap slice is 4B-aligned
                    imgA, imgB = {}, {}
                    for j in range(-WIN, WIN + 1):
                        tA = io.tile([128, W + 2 * WIN], BF, tag=f"imgA{j}")
                        nc.sync.dma_start(
                            out=tA[:],
                            in_=pp3[b, r0 + PAD + j:r0 + PAD + j + 128,
                                    PAD - WIN:PAD + WIN + W])
                        imgA[j] = tA
                        tB = io.tile([128, W + 2 * WIN], BF, tag=f"imgB{j}")
                        nc.sync.dma_start(
                            out=tB[:],
                            in_=pp3[b, r0 + PAD + j:r0 + PAD + j + 128,
                                    PAD - WIN + 1:PAD + WIN + W + 1])
                        imgB[j] = tB
                    dH = io.tile([128, F], FP, tag="dH")
                    nc.sync.dma_start(out=dH[:], in_=flow4[b, 0, r0:r0 + 128, :])
                    dW = io.tile([128, F], FP, tag="dW")
                    nc.sync.dma_start(out=dW[:], in_=flow4[b, 1, r0:r0 + 128, :])

                    # tent weights (Act engine) straight from fp32 flow:
                    # q_k = relu(1 - |R - k|)
                    def tents(R_t, pfx):
                        q = {}
                        for k in range(-WIN, WIN + 1):
                            u = wq.tile([128, F], BF, tag=f"{pfx}u")
                            a.activation(out=u[:], in_=R_t[:], func=AF.Abs,
                                         bias=bias_ap[k][:, 0:1], scale=1.0)
                            qk = wq.tile([128, F], BF, tag=f"{pfx}q{k}")
                            a.activation(out=qk[:], in_=u[:], func=AF.Relu,
                                         bias=1.0, scale=-1.0)
                            q[k] = qk
                        return q

                    qW = tents(dW, "w")
                    qH = tents(dH, "h")

                    def tap_src(j, k):
                        o = k + WIN
                        if o % 2 == 0:
                            return imgA[j][:, o:o + F]
                        return imgB[j][:, o - 1:o - 1 + F]

                    # horizontally-lerped candidate rows (pairwise add tree)
                    HL = {}
                    for j in range(-WIN, WIN + 1):
                        eng = g if j in POOL_ROWS else v
                        sfx = "g" if eng is g else "v"
                        hl = wk.tile([128, F], BF, tag=f"HL{j}")
                        ta = wk.tile([128, F], BF, tag=f"ta{sfx}")
                        tb = wk.tile([128, F], BF, tag=f"tb{sfx}")
                        eng.tensor_tensor(out=ta[:], in0=qW[-WIN][:],
                                          in1=tap_src(j, -WIN), op=AL.mult)
                        eng.tensor_tensor(out=tb[:], in0=qW[-WIN + 1][:],
                                          in1=tap_src(j, -WIN + 1), op=AL.mult)
                        eng.tensor_tensor(out=hl[:], in0=ta[:], in1=tb[:],
                                          op=AL.add)
                        k = -WIN + 2
                        while k <= WIN:
                            if k + 1 <= WIN:
                                ta = wk.tile([128, F], BF, tag=f"ta{sfx}")
                                tb = wk.tile([128, F], BF, tag=f"tb{sfx}")
                                pr = wk.tile([128, F], BF, tag=f"pr{sfx}")
                                eng.tensor_tensor(out=ta[:], in0=qW[k][:],
                                                  in1=tap_src(j, k), op=AL.mult)
                                eng.tensor_tensor(out=tb[:], in0=qW[k + 1][:],
                                                  in1=tap_src(j, k + 1),
                                                  op=AL.mult)
                                eng.tensor_tensor(out=pr[:], in0=ta[:],
                                                  in1=tb[:], op=AL.add)
                                eng.tensor_tensor(out=hl[:], in0=hl[:],
                                                  in1=pr[:], op=AL.add)
                                k += 2
                            else:
                                ta = wk.tile([128, F], BF, tag=f"ta{sfx}")
                                eng.tensor_tensor(out=ta[:], in0=qW[k][:],
                                                  in1=tap_src(j, k), op=AL.mult)
                                eng.tensor_tensor(out=hl[:], in0=hl[:],
                                                  in1=ta[:], op=AL.add)
                                k += 1
                        HL[j] = hl

                    # vertical tent combine (DVE); final add in fp32
                    vacc = wk.tile([128, F], BF, tag="vacc")
                    v.tensor_tensor(out=vacc[:], in0=qH[-WIN][:],
                                    in1=HL[-WIN][:], op=AL.mult)
                    outT = wk.tile([128, F], FP, tag="outT")
                    for j in range(-WIN + 1, WIN + 1):
                        tv = wk.tile([128, F], BF, tag="tav")
                        v.tensor_tensor(out=tv[:], in0=qH[j][:], in1=HL[j][:],
                                        op=AL.mult)
                        if j < WIN:
                            v.tensor_tensor(out=vacc[:], in0=vacc[:],
                                            in1=tv[:], op=AL.add)
                        else:
                            v.tensor_tensor(out=outT[:], in0=vacc[:],
                                            in1=tv[:], op=AL.add)
                    nc.sync.dma_start(out=out3[b, r0:r0 + 128, :], in_=outT[:])

        # ---- sparse fixup (batched indirect DMA) ----
        NCH = nout // 128
        with tc.tile_pool(name="fix", bufs=1) as fx:
            def load_aux(d, dt, name):
                t = fx.tile([128, NCH], dt, tag=name)
                nc.sync.dma_start(
                    out=t[:], in_=d.ap().rearrange("(p f) -> p f", p=128))
                return t

            opos_s = load_aux(opos_d, I32, "opos")
            odh_s = load_aux(odh_d, I32, "odh")
            odw_s = load_aux(odw_d, I32, "odw")
            oh_s = load_aux(oh_d, FP, "oh")
            ow_s = load_aux(ow_d, FP, "ow")
            obase_s = load_aux(obase_d, FP, "obase")

            dhv = fx.tile([128, NCH], FP, tag="dhv")
            dwv = fx.tile([128, NCH], FP, tag="dwv")
            for ch in range(NCH):
                g.indirect_dma_start(
                    out=dhv[:, ch:ch + 1], out_offset=None,
                    in_=flowf[:, None],
                    in_offset=IndirectOffsetOnAxis(ap=odh_s[:, ch:ch + 1],
                                                   axis=0))
                g.indirect_dma_start(
                    out=dwv[:, ch:ch + 1], out_offset=None,
                    in_=flowf[:, None],
                    in_offset=IndirectOffsetOnAxis(ap=odw_s[:, ch:ch + 1],
                                                   axis=0))

            def floor_frac(dv, pfx):
                """floor(R) and (floor(R)+1) - R over R in [FLOOR_LO, FLOOR_HI+1)."""
                St = fx.tile([128, NCH], FP, tag=f"{pfx}S")
                gt = fx.tile([128, NCH], FP, tag=f"{pfx}g")
                v.tensor_scalar(out=St[:], in0=dv[:],
                                scalar1=float(FLOOR_LO + 1), scalar2=None,
                                op0=AL.is_ge)
                for s in range(FLOOR_LO + 2, FLOOR_HI + 1):
                    v.tensor_scalar(out=gt[:], in0=dv[:], scalar1=float(s),
                                    scalar2=None, op0=AL.is_ge)
                    v.tensor_tensor(out=St[:], in0=St[:], in1=gt[:], op=AL.add)
                fl = fx.tile([128, NCH], FP, tag=f"{pfx}fl")
                v.tensor_scalar(out=fl[:], in0=St[:], scalar1=float(FLOOR_LO),
                                scalar2=None, op0=AL.add)
                dd = fx.tile([128, NCH], FP, tag=f"{pfx}dd")
                v.tensor_scalar(out=dd[:], in0=fl[:], scalar1=1.0,
                                scalar2=None, op0=AL.add)
                v.tensor_tensor(out=dd[:], in0=dd[:], in1=dv[:], op=AL.subtract)
                return fl, dd

            flh, ddh = floor_frac(dhv, "fh")
            flw, ddw = floor_frac(dwv, "fw")

            # addr = obase + (oh + floor_h + PAD)*PP + (ow + floor_w + PAD)
            rowp = fx.tile([128, NCH], FP, tag="rowp")
            v.tensor_tensor(out=rowp[:], in0=oh_s[:], in1=flh[:], op=AL.add)
            v.tensor_scalar(out=rowp[:], in0=rowp[:], scalar1=float(PAD),
                            scalar2=float(PP), op0=AL.add, op1=AL.mult)
            colp = fx.tile([128, NCH], FP, tag="colp")
            v.tensor_tensor(out=colp[:], in0=ow_s[:], in1=flw[:], op=AL.add)
            v.tensor_scalar(out=colp[:], in0=colp[:], scalar1=float(PAD),
                            scalar2=None, op0=AL.add)
            af = fx.tile([128, NCH], FP, tag="af")
            v.tensor_tensor(out=af[:], in0=rowp[:], in1=colp[:], op=AL.add)
            v.tensor_tensor(out=af[:], in0=af[:], in1=obase_s[:], op=AL.add)

            # corner gathers: one descriptor per partition moving a contiguous
            # 2-element row (v00,v10) resp. (v01,v11)
            vals = {}
            afo = fx.tile([128, NCH], FP, tag="afo")
            for (rn, doff) in (("n", 0.0), ("s", float(PP))):
                ai = fx.tile([128, NCH], I32, tag=f"ai{rn}")
                if doff == 0.0:
                    v.tensor_copy(out=ai[:], in_=af[:])
                else:
                    v.tensor_scalar(out=afo[:], in0=af[:], scalar1=doff,
                                    scalar2=None, op0=AL.add)
                    v.tensor_copy(out=ai[:], in_=afo[:])
                vt16 = fx.tile([128, 2 * NCH], BF, tag=f"{rn}h")
                for ch in range(NCH):
                    g.indirect_dma_start(
                        out=vt16[:, 2 * ch:2 * ch + 2], out_offset=None,
                        in_=ppf[:, None],
                        in_offset=IndirectOffsetOnAxis(ap=ai[:, ch:ch + 1],
                                                       axis=0))
                vt = fx.tile([128, 2 * NCH], FP, tag=f"{rn}f")
                v.tensor_copy(out=vt[:], in_=vt16[:])
                vals[rn] = vt
            vals = {"v00": vals["n"][:, 0::2], "v10": vals["n"][:, 1::2],
                    "v01": vals["s"][:, 0::2], "v11": vals["s"][:, 1::2]}

            # blend: out = v00*dh*dw + v10*dh*(1-dw) + v01*(1-dh)*dw
            #            + v11*(1-dw)*(1-dh)   with dh=ddh, dw=ddw
            omw = fx.tile([128, NCH], FP, tag="omw")
            v.tensor_scalar(out=omw[:], in0=ddw[:], scalar1=-1.0, scalar2=1.0,
                            op0=AL.mult, op1=AL.add)
            omh = fx.tile([128, NCH], FP, tag="omh")
            v.tensor_scalar(out=omh[:], in0=ddh[:], scalar1=-1.0, scalar2=1.0,
                            op0=AL.mult, op1=AL.add)
            wt = fx.tile([128, NCH], FP, tag="wtf")
            accf = fx.tile([128, NCH], FP, tag="accf")
            t3 = fx.tile([128, NCH], FP, tag="t3")
            v.tensor_tensor(out=wt[:], in0=ddh[:], in1=ddw[:], op=AL.mult)
            v.tensor_tensor(out=accf[:], in0=vals["v00"], in1=wt[:],
                            op=AL.mult)
            v.tensor_tensor(out=wt[:], in0=ddh[:], in1=omw[:], op=AL.mult)
            v.tensor_tensor(out=t3[:], in0=vals["v10"], in1=wt[:], op=AL.mult)
            v.tensor_tensor(out=accf[:], in0=accf[:], in1=t3[:], op=AL.add)
            v.tensor_tensor(out=wt[:], in0=omh[:], in1=ddw[:], op=AL.mult)
            v.tensor_tensor(out=t3[:], in0=vals["v01"], in1=wt[:], op=AL.mult)
            v.tensor_tensor(out=accf[:], in0=accf[:], in1=t3[:], op=AL.add)
            v.tensor_tensor(out=wt[:], in0=omw[:], in1=omh[:], op=AL.mult)
            v.tensor_tensor(out=t3[:], in0=vals["v11"], in1=wt[:], op=AL.mult)
            v.tensor_tensor(out=accf[:], in0=accf[:], in1=t3[:], op=AL.add)

            # scatter: [128,1] per chunk — the only shape validated on HW
            for ch in range(NCH):
                g.indirect_dma_start(
                    out=outf[:, None],
                    out_offset=IndirectOffsetOnAxis(ap=opos_s[:, ch:ch + 1],
                                                    axis=0),
                    in_=accf[:, ch:ch + 1], in_offset=None)

    nc.compile()
    return nc


_PROGRAM_CACHE = {}


def _get_program(nout):
    if nout not in _PROGRAM_CACHE:
        _PROGRAM_CACHE[nout] = _build_program(nout)
    return _PROGRAM_CACHE[nout]


def _host_metadata(dH, dW):
    """Outlier positions for one image, mirroring the reference fp32 math."""
    h = (np.arange(H, dtype=f32)[:, None] * np.ones((1, W), f32))
    w = (np.ones((H, 1), f32) * np.arange(W, dtype=f32)[None, :])
    Rh = (((dH + h).astype(f32) + f32(1.0)).astype(f32)
          - (h + f32(1.0)).astype(f32)).astype(f32)
    Rw = (((dW + w).astype(f32) + f32(1.0)).astype(f32)
          - (w + f32(1.0)).astype(f32)).astype(f32)
    lim = f32(WIN) - MARGIN
    outl = (np.abs(Rh) >= lim) | (np.abs(Rw) >= lim)
    oy, ox = np.where(outl)
    return oy.astype(np.int64), ox.astype(np.int64)


def _prepare(input1, input2):
    """Build (or fetch) the program and the per-core input maps."""
    input1 = np.asarray(input1)
    input2 = np.asarray(input2)
    assert input1.shape == (B, 1, H, W) and input2.shape == (B, 2, H, W)

    metas = []
    max_n = 1
    for c in range(NCORES):
        rows = []
        for bl in range(BPC):
            bglob = c * BPC + bl
            oy, ox = _host_metadata(input2[bglob, 0], input2[bglob, 1])
            rows.append((bl, oy, ox))
        n = sum(len(oy) for _, oy, _ in rows)
        max_n = max(max_n, n)
        metas.append(rows)
    nout = max(128, ((max_n + 127) // 128) * 128)

    nc = _get_program(nout)

    in_maps = []
    for c in range(NCORES):
        imgs = input1[c * BPC:(c + 1) * BPC, 0]
        flow = input2[c * BPC:(c + 1) * BPC]
        opos = np.full(nout, BPC * HW, np.int32)
        odh = np.zeros(nout, np.int32)
        odw = np.full(nout, HW, np.int32)
        oh = np.zeros(nout, f32)
        ow = np.zeros(nout, f32)
        obase = np.zeros(nout, f32)
        k = 0
        for bl, oy, ox in metas[c]:
            n = len(oy)
            opos[k:k + n] = (bl * HW + oy * W + ox).astype(np.int32)
            odh[k:k + n] = (bl * 2 * HW + oy * W + ox).astype(np.int32)
            odw[k:k + n] = (bl * 2 * HW + HW + oy * W + ox).astype(np.int32)
            oh[k:k + n] = oy.astype(f32)
            ow[k:k + n] = ox.astype(f32)
            obase[k:k + n] = f32(bl * PP * PP)
            k += n
        in_maps.append({
            "img": np.ascontiguousarray(imgs),
            "flow": np.ascontiguousarray(flow.reshape(-1)),
            "opos": opos, "odh": odh, "odw": odw,
            "oh": oh, "ow": ow, "obase": obase,
        })

    return nc, in_maps


def _assemble(results):
    out = np.empty((B, 1, H, W), f32)
    for c in range(NCORES):
        o = results[c]["out"][:BPC * HW].reshape(BPC, H, W)
        out[c * BPC:(c + 1) * BPC, 0] = o
    return out


def kernel(input1, input2):
    nc, in_maps = _prepare(input1, input2)
    res = run_bass_kernel_spmd(nc, in_maps, core_ids=list(range(NCORES)))
    return _assemble(res.results)
